# revision 1
# baseline (speedup 1.0000x reference)
"""GQA kernel for Trainium2, 8 NeuronCores.

Sharding: (batch x kv-head-pair). Cores 0-3 handle batch 0, cores 4-7
batch 1; core (bi, j) owns KV heads 2j, 2j+1 (Q heads 8j..8j+7).
Row-parallel Wo: attention outputs are AllGathered per 512-seq chunk
and each core computes a 512-column slice of the final output.

Seq chunks are processed 3->2->1->0 so the expensive collectives fire
early; K/V projections for all chunks plus Q(3) run as a dense PE
preamble, while Q(2,1,0) and the gathered chunks' output projections
are interleaved as fillers into the ACT-paced attention stream.

Host pre-transposes and casts all operands to bf16 (xT, WqT, WkT, WvT,
WoT), so the device does no PE transposes and never loads the mask
(causality is reproduced with an on-device triangular constant).

B=2, S=2048, H=2048, NH=32, NKV=8, HD=64. Matmuls bf16 (f32 PSUM),
softmax statistics f32. Causal structure: fully-masked kv tiles
skipped, AV matmuls column-trimmed around the diagonal, diagonal
128x128 blocks masked multiplicatively with a triangular constant.
"""
import numpy as np
import ml_dtypes

import concourse.bass as bass
import concourse.tile as tile
from concourse import mybir
from concourse.bass_utils import run_bass_kernel_spmd
from concourse.masks import make_upper_triangular

B, S, H = 2, 2048, 2048
NH, NKV, HD = 32, 8, 64
SCALE = HD ** -0.5
F32 = mybir.dt.float32
BF16 = mybir.dt.bfloat16

_program_cache = {}
TRACE = False
LAST_RESULTS = None


def _build_program():
    nc = bass.Bass("TRN2", target_bir_lowering=False, debug=False, num_devices=8)

    xt_in = nc.dram_tensor("xt", [H, S], BF16, kind="ExternalInput").ap()
    wqt_in = nc.dram_tensor("wqt", [H, 512], BF16, kind="ExternalInput").ap()
    wkt_in = nc.dram_tensor("wkt", [H, 128], BF16, kind="ExternalInput").ap()
    wvt_in = nc.dram_tensor("wvt", [H, 128], BF16, kind="ExternalInput").ap()
    wot_in = nc.dram_tensor("wot", [2048, 512], BF16, kind="ExternalInput").ap()
    out_ext = nc.dram_tensor("out_part", [S, 512], F32, kind="ExternalOutput").ap()

    with tile.TileContext(nc) as tc:
        import contextlib
        with (
            tc.tile_pool(name="dram", bufs=1, space="DRAM") as dram,
        ):
          with contextlib.ExitStack() as ctx:
            consts = ctx.enter_context(tc.tile_pool(name="consts", bufs=1))
            wpool = ctx.enter_context(tc.tile_pool(name="wpool", bufs=1))
            qkv = ctx.enter_context(tc.tile_pool(name="qkv", bufs=1))
            ps = ctx.enter_context(tc.tile_pool(name="ps", bufs=2, space="PSUM"))
            av_pool = ctx.enter_context(tc.tile_pool(name="av_pool", bufs=2, space="PSUM"))
            probs_pool = ctx.enter_context(tc.tile_pool(name="probs", bufs=4))
            smalls = ctx.enter_context(tc.tile_pool(name="smalls", bufs=4))

            # causal triangle: tri[kv, q] = 1 if kv <= q else 0
            tri = consts.tile([128, 128], BF16)
            make_upper_triangular(nc, tri, val=1.0, diag=True)

            # ---- persistent sbuf (all pre-transposed bf16 from host) ----
            xT = wpool.tile([128, 16, S], BF16)       # [h%128, h//128, s]
            wqT = wpool.tile([128, 16, 512], BF16)    # [h%128, h//128, qd]
            wkT = wpool.tile([128, 16, 128], BF16)
            wvT = wpool.tile([128, 16, 128], BF16)
            woT = wpool.tile([128, 16, 512], BF16)    # [dg%128, dg//128, o]
            nc.sync.dma_start(out=wkT[:], in_=wkt_in.rearrange("(i p) d -> p i d", p=128))
            nc.sync.dma_start(out=wvT[:], in_=wvt_in.rearrange("(i p) d -> p i d", p=128))
            for cc_ in range(4):
                for hh in range(2):
                    nc.sync.dma_start(
                        out=xT[:, 8 * hh:8 * (hh + 1), 512 * cc_:512 * (cc_ + 1)],
                        in_=xt_in[1024 * hh:1024 * (hh + 1),
                                  512 * cc_:512 * (cc_ + 1)]
                            .rearrange("(i p) s -> p i s", p=128))
            nc.sync.dma_start(out=wqT[:], in_=wqt_in.rearrange("(i p) d -> p i d", p=128))
            nc.sync.dma_start(out=woT[:], in_=wot_in.rearrange("(i p) o -> p i o", p=128))

            qt_sb = qkv.tile([128, 4, S], BF16)       # [64*half+d, pair m, s]
            kt_sb = qkv.tile([128, S], BF16)          # [64*kh+d, skv]
            v_sb = qkv.tile([128, 16, 130], BF16)     # [skv%128, skv//128, V0|1|V1|1]
            ot_sb = qkv.tile([128, 4, S], BF16)       # [64*half+d, pair m, s]
            nc.vector.memset(v_sb, 1.0)

            ogpool = ctx.enter_context(tc.tile_pool(name="ogpool", bufs=2))
            outst = ctx.enter_context(tc.tile_pool(name="outst", bufs=3))
            recip_dram = dram.tile([32, 512], F32)

            def proj_piece(c, kind, idx):
                """One projection piece for chunk c: Q pair, K, or V tile."""
                cs = slice(512 * c, 512 * (c + 1))
                if kind == "q":
                    qp = ps.tile([128, 512], F32, tag="pj", name=f"qp_{c}_{idx}")
                    for i in range(16):
                        nc.tensor.matmul(qp[:], wqT[:, i, 128 * idx:128 * (idx + 1)],
                                         xT[:, i, cs], start=(i == 0), stop=(i == 15))
                    nc.vector.tensor_copy(qt_sb[:, idx, cs], qp[:])
                elif kind == "k":
                    kp = ps.tile([128, 512], F32, tag="pj", name=f"kp_{c}")
                    for i in range(16):
                        nc.tensor.matmul(kp[:], wkT[:, i, :], xT[:, i, cs],
                                         start=(i == 0), stop=(i == 15))
                    nc.vector.tensor_copy(kt_sb[:, cs], kp[:])
                else:
                    t = 4 * c + idx
                    vp = ps.tile([128, 512], F32, tag="pj", name=f"vp_{t}")
                    for i in range(16):
                        nc.tensor.matmul(
                            vp[:, 0:128],
                            xT[:, i, 512 * c + 128 * idx:512 * c + 128 * (idx + 1)],
                            wvT[:, i, :], start=(i == 0), stop=(i == 15))
                    nc.vector.tensor_copy(v_sb[:, t, 0:64], vp[:, 0:64])
                    nc.vector.tensor_copy(v_sb[:, t, 65:129], vp[:, 64:128])

            RG = [[0, 1, 2, 3], [4, 5, 6, 7]]

            ccis = {}

            def stage_m(c, m):
                """Stage one m-pair of chunk c's output for the gather."""
                cs = slice(512 * c, 512 * (c + 1))
                if c not in ccis:
                    ccis[c] = dram.tile([4, 128, 512], BF16, tag="cci", bufs=2,
                                        name=f"cci_{c}")
                nc.gpsimd.dma_start(out=ccis[c][m], in_=ot_sb[:, m, cs])

            def gather_chunk(c):
                """AllGather chunk c's staged output, start og load."""
                cci = ccis[c]
                cco = dram.tile([2048, 512], BF16, tag="cco", bufs=2,
                                name=f"cco_{c}")
                nc.gpsimd.collective_compute(
                    "AllGather", mybir.AluOpType.bypass, replica_groups=RG,
                    ins=[cci.opt()], outs=[cco.opt()])
                og = ogpool.tile([128, 16, 512], BF16, tag="og", name=f"og_{c}")
                for q_ in range(4):
                    nc.sync.dma_start(
                        out=og[:, 4 * q_:4 * (q_ + 1), :],
                        in_=cco[512 * q_:512 * (q_ + 1), :]
                            .rearrange("(i p) s -> p i s", p=128))
                ogs[c] = og

            def out_proj_piece(c, st):
                """One 128-seq-row block of the output projection of chunk c."""
                og = ogs[c]
                op = ps.tile([128, 512], F32, tag="pj", name=f"op_{c}_{st}")
                for i in range(16):
                    nc.tensor.matmul(op[:], og[:, i, 128 * st:128 * (st + 1)],
                                     woT[:, i, :], start=(i == 0), stop=(i == 15))
                ost = outst.tile([128, 512], F32, tag="ost", name=f"ost_{c}_{st}")
                nc.vector.tensor_copy(ost[:], op[:])
                nc.sync.dma_start(
                    out=out_ext[512 * c + 128 * st:512 * c + 128 * (st + 1), :],
                    in_=ost[:])

            ogs = {}
            # ---- preamble: K/V for all chunks (small weights arrive first),
            # then Q for chunk 3 (processed first) ----
            for c_ in range(4):
                proj_piece(c_, "k", 0)
                for r in range(4):
                    proj_piece(c_, "v", r)
            for m in range(4):
                proj_piece(3, "q", m)

            # chunks processed 3 -> 0: the expensive gathers are issued early
            # and each chunk's out-proj fills the NEXT (smaller) chunk's
            # ACT-paced attention. Q projections of chunks 2,1,0 fill chunk 3.
            for ci, c in enumerate((3, 2, 1, 0)):
                cs = slice(512 * c, 512 * (c + 1))
                # ---- filler schedule: one independent PE piece per slot ----
                fillers = []
                if c == 3:
                    for qc in (2, 1, 0):
                        fillers += [("proj", qc, "q", m_) for m_ in range(4)]
                elif c >= 1:
                    # out-proj of the previously processed (larger) chunk;
                    # chunk 0's attention is too short for gather(1) to land.
                    fillers += [("oproj", c + 1, st) for st in range(4)]
                total_groups = 4 * (2 * c + 2)
                start_g = (7 * total_groups) // 8 if c != 3 else 2
                posns = {}
                for k, f in enumerate(fillers):
                    p_ = start_g + (k * (total_groups - start_g)) // max(len(fillers), 1)
                    posns.setdefault(min(p_, total_groups - 1), []).append(f)
                gidx = 0

                def emit_fillers():
                    for f in posns.get(gidx, ()):
                        if f[0] == "proj":
                            proj_piece(f[1], f[2], f[3])
                        else:
                            out_proj_piece(f[1], f[2])

                # ---- attention for this s-chunk ----
                ntile = 4 * c + 4
                for m in range(4):
                    avA = av_pool.tile([65, 512], F32, tag="av", name=f"avA_{c}_{m}")
                    avB = av_pool.tile([65, 512], F32, tag="av", name=f"avB_{c}_{m}")
                    for g in range(ntile // 2):
                        sp = []
                        pr = []
                        for half in range(2):
                            s_ = ps.tile([128, 1024], F32, tag="mm",
                                         name=f"sp_{c}_{m}_{g}_{half}")
                            p_ = probs_pool.tile([128, 1024], BF16, tag="pr",
                                                 name=f"pr_{c}_{m}_{g}_{half}")
                            sp.append(s_)
                            pr.append(p_)
                        for tt in range(2):
                            t = 2 * g + tt
                            for half in range(2):
                                nc.tensor.matmul(
                                    sp[half][:, 512 * tt:512 * (tt + 1)],
                                    kt_sb[64 * half:64 * half + 64, 128 * t:128 * (t + 1)],
                                    qt_sb[64 * half:64 * half + 64, m, cs],
                                    start=True, stop=True,
                                    tile_position=(64 * half, 0))
                        for half in range(2):
                            nc.scalar.activation(pr[half][:], sp[half][:],
                                                 mybir.ActivationFunctionType.Exp,
                                                 scale=SCALE)
                        for tt in range(2):
                            t = 2 * g + tt
                            d = t - 4 * c  # >= 0 on the diagonal band
                            if d >= 0:
                                for half in range(2):
                                    blk = slice(512 * tt + 128 * d, 512 * tt + 128 * (d + 1))
                                    nc.vector.tensor_mul(pr[half][:, blk], pr[half][:, blk],
                                                         tri[:])
                            lo = 128 * d if d > 0 else 0
                            for half, av in ((0, avA), (1, avB)):
                                nc.tensor.matmul(
                                    av[:, lo:512],
                                    v_sb[:, t, 65 * half:65 * half + 65],
                                    pr[half][:, 512 * tt + lo:512 * (tt + 1)],
                                    start=(t == 0), stop=(t == ntile - 1),
                                    skip_group_check=True)
                        emit_fillers()
                        gidx += 1
                    # ---- divide by rowsum; copy av to SBUF first so the
                    # PSUM slot frees without waiting on the recip chain ----
                    for half, av in ((0, avA), (1, avB)):
                        u = 8 * c + 2 * m + half
                        avs = smalls.tile([65, 512], F32, tag="avs", bufs=3, name=f"avs_{u}")
                        nc.vector.tensor_copy(avs[:], av[:])
                        # 1/x = exp(-ln x): both fns share one ACT table
                        # set, moving the 3.3us DVE reciprocal to ScalarE
                        lnr = smalls.tile([1, 512], F32, tag="rc", bufs=2,
                                          name=f"lnr_{u}")
                        nc.scalar.activation(lnr[:], avs[64:65, :],
                                             mybir.ActivationFunctionType.Ln)
                        rc = smalls.tile([1, 512], F32, tag="rc", bufs=2, name=f"rc_{u}")
                        nc.scalar.activation(rc[:], lnr[:],
                                             mybir.ActivationFunctionType.Exp,
                                             scale=-1.0)
                        nc.sync.dma_start(out=recip_dram[u:u + 1, :], in_=rc[:])
                        bcb = smalls.tile([64, 512], F32, tag="bc", bufs=2, name=f"bc_{u}")
                        nc.gpsimd.dma_start(
                            out=bcb[:],
                            in_=recip_dram[u:u + 1, :].partition_broadcast(64))
                        nc.vector.tensor_mul(
                            ot_sb[64 * half:64 * half + 64, m, cs],
                            avs[0:64, :], bcb[:])
                    stage_m(c, m)

                gather_chunk(c)

            # tail: out-proj of the last two chunks (their fillers never ran)
            for st in range(4):
                out_proj_piece(1, st)
            for st in range(4):
                out_proj_piece(0, st)

    _split_excess_waits(nc)
    return nc


def _split_excess_waits(nc, cap=1):
    """Walrus allows few sync-wait slots per instruction; move excess waits
    onto same-engine NoOps placed immediately before (program order keeps
    semantics)."""
    nid = [0]
    for fn in nc.m.functions:
        for bb in fn.blocks:
            insts = list(bb.instructions)
            out = []
            for inst in insts:
                si = getattr(inst, "sync_info", None)
                waits = list(si.on_wait) if si and si.on_wait else []
                if len(waits) > cap:
                    keep = waits[:cap]
                    rest = waits[cap:]
                    while rest:
                        chunk, rest = rest[:cap], rest[cap:]
                        nid[0] += 1
                        nop = mybir.InstNoOp(
                            name=f"waitsplit-{nid[0]}", engine=inst.engine,
                            ins=[], outs=[], bass_nofuse=True,
                            sync_info=mybir.SyncInfo(on_wait=chunk, on_update=[]))
                        out.append(nop)
                    si.on_wait = keep
                out.append(inst)
            bb.instructions[:] = out


def _perm():
    """gathered-OT row dg -> original Wo column (within a 4-core group).

    dg = 512*src_rank + 128*m + 64*half + d  <->  q head 8*src_rank + m + 4*half,
    head dim d; Wo column = 64*head + d.
    """
    p = np.zeros(2048, np.int64)
    for g in range(2048):
        rank, r = divmod(g, 512)
        m, rr = divmod(r, 128)
        half, d = divmod(rr, 64)
        h = 8 * rank + m + 4 * half
        p[g] = 64 * h + d
    return p


def kernel(hidden_states, attention_mask, Wq, Wk, Wv, Wo):
    hidden_states = np.asarray(hidden_states, np.float32)
    Wq = np.asarray(Wq, np.float32); Wk = np.asarray(Wk, np.float32)
    Wv = np.asarray(Wv, np.float32); Wo = np.asarray(Wo, np.float32)

    if "nc" not in _program_cache:
        _program_cache["nc"] = _build_program()
    nc = _program_cache["nc"]

    bf = ml_dtypes.bfloat16
    perm = _perm()
    xt = [np.ascontiguousarray(hidden_states[bi].T.astype(bf)) for bi in range(B)]

    in_maps = []
    for core in range(8):
        bi, j = divmod(core, 4)
        rows = []
        for m in range(4):
            rows.append(Wq[64 * (8 * j + m):64 * (8 * j + m) + 64])
            rows.append(Wq[64 * (8 * j + 4 + m):64 * (8 * j + 4 + m) + 64])
        wq = np.concatenate(rows, 0)                       # [512, H]
        in_maps.append({
            "xt": xt[bi],
            "wqt": np.ascontiguousarray(wq.T.astype(bf)),
            "wkt": np.ascontiguousarray(Wk[128 * j:128 * (j + 1)].T.astype(bf)),
            "wvt": np.ascontiguousarray(Wv[128 * j:128 * (j + 1)].T.astype(bf)),
            # [2048 dg, 512 o]: own 512 output cols, rows in gathered-OT order
            "wot": np.ascontiguousarray(Wo[512 * j:512 * (j + 1), :][:, perm].T
                                        .astype(bf)),
        })

    kw = {}
    if TRACE:
        kw = dict(trace=True, trace_cores=[0])
    res = run_bass_kernel_spmd(nc, in_maps, list(range(8)), **kw)
    global LAST_RESULTS
    LAST_RESULTS = res

    out = np.zeros((B, S, H), np.float32)
    for core in range(8):
        bi, j = divmod(core, 4)
        out[bi, :, 512 * j:512 * (j + 1)] = res.results[core]["out_part"]
    return out


if __name__ == "__main__":
    ins = {
        "hidden_states": np.random.randn(B, S, H).astype(np.float32),
        "attention_mask": np.zeros((B, 1, S, S), np.float32),
        "Wq": np.random.randn(2048, H).astype(np.float32) * H ** -0.5,
        "Wk": np.random.randn(512, H).astype(np.float32) * H ** -0.5,
        "Wv": np.random.randn(512, H).astype(np.float32) * H ** -0.5,
        "Wo": np.random.randn(H, 2048).astype(np.float32) * H ** -0.5,
    }
    o = kernel(**ins)
    print("ran", o.shape, o.dtype)

